# revision 47
# baseline (speedup 1.0000x reference)
"""Multi-head attention (B=4, P=2048, D=1024, H=16) on 8 TRN2 NeuronCores.

Sharding: tensor-parallel over heads (2 heads per core). Each core computes
qkv for its heads, full attention for its heads, and a partial output
projection (rows of w_proj for its heads). Partials are summed on host.

v15: single dense PE instruction stream, paced by the ACT engine's exp
throughput. Per batch the attention sweep (scores -> exp -> attn*V)
interleaves two paced filler queues: q_pre (batch b+1's qkv, must finish
before the next sweep starts) and q_post (normalization + output
projection, deadline-free, carries across sweep boundaries). Every
matmul contracts K=128 so the PE array never reconfigures its tile mode:
score stationaries are zero-padded per head (kt holds h0 blocks with
rows 64:128 zeroed and h1 blocks with rows 0:64 zeroed, streamed against
the full 2-head qt), and the denominator broadcast uses a K=65 matrix
whose only nonzero row is the ones row. v is computed directly in
[keys, d] layout (stationary = x chunks) so no PE transposes are needed.
The output projection contracts K=128 in one matmul per tile via the
packed oTn2 [128, P] tile, whose halves are written by cross-partition
DVE multiplies. Unnormalized o^T is stored bf16 so the broadcast
matmul's moving operand streams at 1 column/cycle. The last batch's
projection drains on alternating PSUM banks with its PSUM-drain copies
split across DVE and ACT (both idle in the tail). Scheduling details
that matter: filler-queue copy units are yield points (so a fresh
accumulation group never WAR-waits right behind its predecessor's
drain), the serial prologue alternates qkv groups between ps_q and the
then-idle scores PSUM pool for the same reason, half the filler budget
lands between the two attn*V chunks of each pair (hiding exp latency),
and extra filler is pulled forward at each segment tail where the PE
would otherwise catch up with the ACT engine's exp backlog.
"""

import numpy as np
import ml_dtypes

import concourse.bass as bass
import concourse.tile as tile
from concourse import bacc, mybir
from concourse import bass_utils

B, P, D = 4, 2048, 1024
H = 16
NCORES = 8
HPC = H // NCORES          # heads per core = 2
d = D // H                 # 64
R = B * P                  # 8192
SCALE = float(d) ** -0.5

F32 = mybir.dt.float32
F32R = mybir.dt.float32r
BF16 = mybir.dt.bfloat16
AF = mybir.ActivationFunctionType

_CACHE = {}


def _build():
    nc = bacc.Bacc("TRN2", target_bir_lowering=False, debug=False,
                   enable_asserts=False)
    xT = nc.dram_tensor("xT", (D, R), BF16, kind="ExternalInput").ap()
    wqkv = nc.dram_tensor("wqkv", (128, 3072), BF16, kind="ExternalInput").ap()
    wproj = nc.dram_tensor("wproj", (128, D), BF16, kind="ExternalInput").ap()
    out = nc.dram_tensor("out", (R, D), F32, kind="ExternalOutput").ap()

    xT3 = xT.rearrange("(kb p) n -> p kb n", p=128)      # [128, 8, 8192]
    out3 = out.rearrange("(r p) n -> p r n", p=128)      # [128, 64, 1024]

    with tile.TileContext(nc) as tc:
        from contextlib import ExitStack
        from collections import deque
        with ExitStack() as ctx:
            p_const = ctx.enter_context(tc.tile_pool(name="const", bufs=1))
            p_w = ctx.enter_context(tc.tile_pool(name="w", bufs=1))
            p_x = ctx.enter_context(tc.tile_pool(name="x", bufs=3))
            p_qk = ctx.enter_context(tc.tile_pool(name="qk", bufs=2))
            p_v = ctx.enter_context(tc.tile_pool(name="v", bufs=2))
            p_e = ctx.enter_context(tc.tile_pool(name="e", bufs=8))
            p_otu = ctx.enter_context(tc.tile_pool(name="otu", bufs=2))
            p_on = ctx.enter_context(tc.tile_pool(name="on", bufs=2))
            p_bc = ctx.enter_context(tc.tile_pool(name="bc", bufs=4))
            p_out = ctx.enter_context(tc.tile_pool(name="o", bufs=2))
            # PSUM: scores 2x[128,1024]=4, attnV 1x[65,1024]=2,
            # qkv 1x[128,512]=1, proj/bc 1x[128,512]=1  -> 8 banks
            ps_s = ctx.enter_context(
                tc.tile_pool(name="pss", bufs=2, space="PSUM"))
            ps_o = ctx.enter_context(
                tc.tile_pool(name="pso", bufs=1, space="PSUM"))
            ps_q = ctx.enter_context(
                tc.tile_pool(name="psq", bufs=1, space="PSUM"))
            ps_p = ctx.enter_context(
                tc.tile_pool(name="psp", bufs=1, space="PSUM"))

            # K=65 broadcast matrix: only row 64 (the denominator row of
            # oTu) is ones, rows 0-63 contribute zero. Keeps the PE array
            # in its 128-row tile mode (no reconfiguration).
            ones65 = p_const.tile([65, 128], BF16)
            nc.vector.memset(ones65[:], 0.0)
            nc.vector.memset(ones65[64:65, :], 1.0)

            # warm the ACT engine's Exp table during the prologue
            wtmp = p_const.tile([64, 64], F32)
            nc.vector.memset(wtmp[:], 0.0)
            wout = p_const.tile([64, 64], BF16)
            nc.scalar.activation(wout[:], wtmp[:], AF.Exp)

            wq_sb = p_w.tile([128, 3072], BF16)
            nc.sync.dma_start(wq_sb[:], wqkv[:])
            wp_sb = p_w.tile([128, D], BF16)
            nc.sync.dma_start(wp_sb[:], wproj[:])

            # warm-up matmuls on scratch data: the PE only reaches its full
            # 2.4 GHz pstate after ~3us of continuous execution, so spin it
            # up during the initial weight/x DMA (whose results nothing
            # here depends on)
            scr = p_const.tile([65, 512], BF16)
            nc.vector.memset(scr[:], 0.5)
            for _ in range(12):
                pw = ps_p.tile([128, 512], F32, tag="p", name="warm")
                nc.tensor.matmul(pw[:], ones65[:], scr[:],
                                 start=True, stop=True)

            # ---- paced filler queues: (cycles_estimate, emit_fn) ----
            q_pre = deque()      # next batch's qkv: hard deadline (sweep end)
            q_post = deque()     # normalize + proj: no deadline, carries over
            done_pre = [0.0]
            done_post = [0.0]

            def emit_q(q, done, target, respect_yield=True):
                while q and done[0] < target:
                    unit = q.popleft()
                    cyc, fn = unit[0], unit[1]
                    fn()
                    done[0] += cyc
                    # copy units are yield points: stop so the consumer's
                    # PSUM-drain overlaps the sweep instead of stalling the
                    # next same-bank matmul right behind it
                    if respect_yield and len(unit) > 2 and unit[2]:
                        break

            def flush(q, done):
                emit_q(q, done, float("inf"), respect_yield=False)

            state = {}

            # ---- qkv stage for one batch, as q_pre units ----
            def queue_qkv(b, prologue=False):
                st = {"xts": {}, "ps": {}}
                state[b] = st
                units = []

                def u_alloc():
                    st["qt"] = p_qk.tile([128, P], BF16, tag="qt",
                                         name=f"qt{b}")
                    # kt zero-padded to K=128 per head: h0 blocks in cols
                    # [0,P) rows 0:64 (rows 64:128 zero), h1 blocks in cols
                    # [P,2P) rows 64:128 (rows 0:64 zero). Scores then run
                    # as uniform 128x128x512 matmuls against the full qt.
                    st["kt"] = p_qk.tile([128, 2 * P], BF16, tag="kt",
                                         name=f"kt{b}")
                    nc.gpsimd.memset(st["kt"][64:128, 0:P], 0.0)
                    nc.gpsimd.memset(st["kt"][0:64, P:2 * P], 0.0)
                    st["vON"] = [
                        p_v.tile([128, 16 * 65], BF16, tag=f"v{h}",
                                 name=f"vON{b}_{h}")
                        for h in range(2)]
                    for h in range(2):
                        ov = st["vON"][h].rearrange(
                            "p (blk w) -> p blk w", w=65)
                        nc.vector.memset(ov[:, :, 64:65], 1.0)

                def u_dma(cc):
                    def fn():
                        xt = p_x.tile([128, 8 * 512], BF16, tag="x",
                                      name=f"xt{b}_{cc}")
                        st["xts"][cc] = xt
                        c = b * 4 + cc
                        nc.sync.dma_start(
                            xt.rearrange("p (kb n) -> p kb n", n=512),
                            xT3[:, :, c * 512:(c + 1) * 512])
                    return (0, fn)

                def u_qk_mm(cc, m, i, alt_ps=False):
                    # kb pair (2i, 2i+1), m in {0,1}
                    def fn():
                        if i == 0:
                            pool = ps_s if alt_ps else ps_q
                            tag = "s" if alt_ps else "q"
                            st["ps"][(cc, m)] = pool.tile(
                                [128, 512], F32, tag=tag, name="qkps")
                        ps = st["ps"][(cc, m)]
                        xt = st["xts"][cc]
                        for kb in (2 * i, 2 * i + 1):
                            col = kb * 384 + m * 128
                            nc.tensor.matmul(
                                ps[:], wq_sb[:, col:col + 128],
                                xt[:, kb * 512:(kb + 1) * 512],
                                start=(kb == 0), stop=(kb == 7))
                    return (1024, fn)

                def u_qk_copy(cc, m):
                    def fn():
                        ps = st["ps"].pop((cc, m))
                        if m == 0:
                            sl = slice(cc * 512, (cc + 1) * 512)
                            nc.vector.tensor_copy(st["qt"][:, sl], ps[:])
                        else:
                            kt = st["kt"]
                            for h in range(2):
                                sl = slice(h * P + cc * 512,
                                           h * P + (cc + 1) * 512)
                                nc.vector.tensor_scalar_mul(
                                    kt[h * 64:(h + 1) * 64, sl],
                                    ps[h * 64:(h + 1) * 64, :], SCALE)
                    return (64, fn, True)

                def u_v_mm(cc, rs, i, alt_ps=False):   # kb quad (4i..4i+3)
                    def fn():
                        if i == 0:
                            pool = ps_s if alt_ps else ps_q
                            tag = "s" if alt_ps else "q"
                            st["ps"][(cc, 2, rs)] = pool.tile(
                                [128, 128], F32, tag=tag, name="vps")
                        ps = st["ps"][(cc, 2, rs)]
                        xt = st["xts"][cc]
                        for kb in range(4 * i, 4 * i + 4):
                            col = kb * 384 + 256
                            nc.tensor.matmul(
                                ps[:],
                                xt[:, kb * 512 + rs * 128:
                                   kb * 512 + rs * 128 + 128],
                                wq_sb[:, col:col + 128],
                                start=(kb == 0), stop=(kb == 7))
                    return (512, fn)

                def u_v_copy(cc, rs):
                    def fn():
                        ps = st["ps"].pop((cc, 2, rs))
                        jb = cc * 4 + rs
                        for h in range(2):
                            nc.vector.tensor_copy(
                                st["vON"][h][:, jb * 65:jb * 65 + 64],
                                ps[:, h * 64:(h + 1) * 64])
                    return (64, fn, True)

                # Prologue runs serially: defer chunk prefetches so the
                # first matmul's inputs (weights + chunk 0) get the full DMA
                # bandwidth. Interleaved batches are paced over a whole
                # sweep: prefetch early so no qkv unit ever waits.
                units.append(u_dma(0))
                units.append((32, u_alloc))
                if not prologue:
                    units.append(u_dma(1))
                # In the serial prologue, alternate qkv accumulation groups
                # between ps_q and the (idle) scores PSUM pool so a group's
                # first matmul never WAR-waits on the previous group's drain
                gi = 0
                for cc in range(4):
                    if prologue:
                        if cc + 1 <= 3:
                            units.append(u_dma(cc + 1))
                    elif cc + 2 <= 3:
                        units.append(u_dma(cc + 2))
                    for m in range(2):
                        ap = prologue and (gi % 2 == 1)
                        gi += 1
                        for i in range(4):
                            units.append(u_qk_mm(cc, m, i, alt_ps=ap))
                        units.append(u_qk_copy(cc, m))
                    for rs in range(4):
                        ap = prologue and (gi % 2 == 1)
                        gi += 1
                        for i in range(2):
                            units.append(u_v_mm(cc, rs, i, alt_ps=ap))
                        units.append(u_v_copy(cc, rs))
                q_pre.extend(units)

            # ---- normalization for one ic2 half, as q_post units ----
            # alt_pool: batches whose proj/bc can alternate into ps_q
            # (only when no qkv units will share ps_q: sweeps 3 and beyond)
            def queue_norm_h(b, ic2, h):
                st = state[b]

                def fn():
                    oTu = st["oTu"][ic2]
                    oTn2 = st["oTn2"]
                    for icc in range(2):
                        sl = slice(ic2 * 1024 + icc * 512,
                                   ic2 * 1024 + (icc + 1) * 512)
                        osl = slice(icc * 512, (icc + 1) * 512)
                        pbc = ps_p.tile([128, 512], F32, tag="p",
                                        name="pbc")
                        nc.tensor.matmul(pbc[:], ones65[:],
                                         oTu[h][:, osl],
                                         start=True, stop=True)
                        bcs = p_bc.tile([64, 512], F32, tag="bc",
                                        name="bcs")
                        nc.vector.reciprocal_approx_fast(bcs[:],
                                                         pbc[0:64, :])
                        nc.vector.tensor_mul(
                            oTn2[h * 64:(h + 1) * 64, sl],
                            oTu[h][0:64, osl], bcs[:])

                if ic2 == 0 and h == 0:
                    def alloc_on():
                        state[b]["oTn2"] = p_on.tile(
                            [128, P], BF16, tag="on", name=f"oTn2_{b}")
                    q_post.append((0, alloc_on))
                q_post.append((1200, fn, True))

            # ---- output projection for one ic2 half, as q_post units ----
            def queue_proj(b, ic2):
                st_o = {}
                alt = (b == 3)

                def u_proj(rr, half):
                    def fn():
                        oTn2 = state[b]["oTn2"]
                        if half == 0:
                            st_o[rr] = p_out.tile([128, 1024], F32, tag="os",
                                                  name="outsb")
                        outsb = st_o[rr]
                        if alt and ic2 == 1:
                            # final drain: every PSUM pool is free, rotate
                            # across four so each slot's mm->copy->mm
                            # round-trip overlaps three others
                            pool, tag = [(ps_p, "p"), (ps_q, "q"),
                                         (ps_s, "s"), (ps_o, "o")][
                                             (rr * 2 + half) % 4]
                        elif alt and (rr * 2 + half) % 2:
                            pool, tag = ps_q, "q"
                        else:
                            pool, tag = ps_p, "p"
                        psp = pool.tile([128, 512], F32, tag=tag, name="pjps")
                        nc.tensor.matmul(
                            psp[:], oTn2[:, rr * 128:(rr + 1) * 128],
                            wp_sb[:, half * 512:(half + 1) * 512],
                            start=True, stop=True)
                        if alt and half == 1:
                            # last batch's tail: split the PSUM-drain copies
                            # across ACT and DVE (both near-idle by then)
                            nc.scalar.copy(
                                outsb[:, half * 512:(half + 1) * 512],
                                psp[:])
                        else:
                            nc.vector.tensor_copy(
                                outsb[:, half * 512:(half + 1) * 512],
                                psp[:])
                        if half == 1:
                            r0 = b * 16 + rr
                            nc.sync.dma_start(
                                out3[:, r0:r0 + 1, :],
                                st_o.pop(rr).rearrange("p (r n) -> p r n",
                                                       n=1024))
                    return (600, fn, True)

                for rr in range(ic2 * 8, ic2 * 8 + 8):
                    for half in range(2):
                        q_post.append(u_proj(rr, half))

            # ---- attention sweep for one batch (paced by ACT exps) ----
            def sweep(b):
                st = state[b]
                qt, kt, vON = st["qt"], st["kt"], st["vON"]
                st["oTu"] = {}
                pre_total = done_pre[0] + sum(u[0] for u in q_pre)
                prog = [0.0]

                def pace(step):
                    prog[0] += step
                    frac = prog[0] / 32.0
                    emit_q(q_pre, done_pre, pre_total * frac)
                    post_total = done_post[0] + sum(u[0] for u in q_post)
                    emit_q(q_post, done_post, post_total * frac)

                def boost(c):
                    # pull filler forward so the segment-tail attn*V pair
                    # doesn't catch up with the ACT engine's exp backlog
                    if q_pre:
                        emit_q(q_pre, done_pre, done_pre[0] + c)
                    else:
                        emit_q(q_post, done_post, done_post[0] + c)

                for ic2 in range(2):
                    q0 = ic2 * 1024
                    oTu = [p_otu.tile([65, P // 2], BF16, tag=f"otu{h}",
                                      name=f"oTu{b}_{ic2}_{h}")
                           for h in range(2)]
                    st["oTu"][ic2] = oTu
                    for h in range(2):
                        es = [None] * 16
                        psos = None
                        for jp in range(8):
                            jbs = (2 * jp, 2 * jp + 1)
                            pts = []
                            for jb in jbs:
                                pss = ps_s.tile([128, 1024], F32, tag="s",
                                                name="pss")
                                kb0 = h * P + jb * 128
                                for hf in range(2):
                                    nc.tensor.matmul(
                                        pss[:, hf * 512:(hf + 1) * 512],
                                        kt[:, kb0:kb0 + 128],
                                        qt[:, q0 + hf * 512:
                                           q0 + (hf + 1) * 512],
                                        start=True, stop=True)
                                pts.append(pss)
                            for jb, pss in zip(jbs, pts):
                                et = p_e.tile([128, 1024], BF16, tag="e",
                                              name="et")
                                nc.scalar.activation(et[:], pss[:], AF.Exp)
                                es[jb] = et
                            if jp == 1:
                                psos = ps_o.tile([65, 1024], F32, tag="o",
                                                 name="psos")
                            if jp >= 1:
                                jb = 2 * jp - 2
                                for hf in range(2):
                                    nc.tensor.matmul(
                                        psos[:, hf * 512:(hf + 1) * 512],
                                        vON[h][:, jb * 65:(jb + 1) * 65],
                                        es[jb][:, hf * 512:(hf + 1) * 512],
                                        start=(jb == 0), stop=False)
                            # filler here gives exp(2jp-1) time to finish
                            # before its attn*V consumer issues
                            pace(0.5)
                            if jp >= 1:
                                jb = 2 * jp - 1
                                for hf in range(2):
                                    nc.tensor.matmul(
                                        psos[:, hf * 512:(hf + 1) * 512],
                                        vON[h][:, jb * 65:(jb + 1) * 65],
                                        es[jb][:, hf * 512:(hf + 1) * 512],
                                        start=False, stop=False)
                            pace(0.5)
                        boost(1000)
                        for hf in range(2):
                            nc.tensor.matmul(
                                psos[:, hf * 512:(hf + 1) * 512],
                                vON[h][:, 14 * 65:15 * 65],
                                es[14][:, hf * 512:(hf + 1) * 512],
                                start=False, stop=False)
                        boost(600)
                        for hf in range(2):
                            nc.tensor.matmul(
                                psos[:, hf * 512:(hf + 1) * 512],
                                vON[h][:, 15 * 65:16 * 65],
                                es[15][:, hf * 512:(hf + 1) * 512],
                                start=False, stop=True)
                        nc.vector.tensor_copy(oTu[h][:], psos[:])
                        queue_norm_h(b, ic2, h)
                    queue_proj(b, ic2)

            # ---- pipeline ----
            queue_qkv(0, prologue=True)
            flush(q_pre, done_pre)             # prologue: qkv(0) serial
            for b in range(B):
                if b + 1 < B:
                    queue_qkv(b + 1)
                sweep(b)
                flush(q_pre, done_pre)         # qkv(b+1) must be done
            flush(q_post, done_post)           # epilogue: tail of proj(3)

    nc.compile()
    return nc


def _in_maps(x, w_qkv, w_proj):
    x2 = np.ascontiguousarray(x.reshape(R, D).T)          # (D, R)
    xbf = x2.astype(ml_dtypes.bfloat16)
    Wq = w_qkv.reshape(D, 3, H, d)
    Wp = w_proj.reshape(H, d, D)
    maps = []
    for c in range(NCORES):
        hs = slice(c * HPC, (c + 1) * HPC)
        # per-core qkv weight shard, columns ordered (qkv, head, d)
        w_shard = np.ascontiguousarray(Wq[:, :, hs, :]).reshape(D, 3 * HPC * d)
        # pre-tile: [p, kb*384 + m*128 + col] = w_shard[kb*128+p, m*128+col]
        wq_pre = np.ascontiguousarray(
            w_shard.reshape(8, 128, 3, 128).transpose(1, 0, 2, 3)
        ).reshape(128, 3072)
        wp_shard = np.ascontiguousarray(Wp[hs]).reshape(HPC * d, D)
        maps.append({
            "xT": xbf,
            "wqkv": np.ascontiguousarray(wq_pre).astype(ml_dtypes.bfloat16),
            "wproj": wp_shard.astype(ml_dtypes.bfloat16),
        })
    return maps


def get_nc():
    if "nc" not in _CACHE:
        _CACHE["nc"] = _build()
    return _CACHE["nc"]


def kernel(x, w_qkv, w_proj, b_proj):
    x = np.asarray(x)
    w_qkv = np.asarray(w_qkv)
    w_proj = np.asarray(w_proj)
    b_proj = np.asarray(b_proj)
    nc = get_nc()
    maps = _in_maps(x, w_qkv, w_proj)
    res = bass_utils.run_bass_kernel_spmd(nc, maps, core_ids=list(range(NCORES)))
    acc = np.zeros((R, D), dtype=np.float64)
    for r in res.results:
        acc += r["out"].astype(np.float64)
    acc += b_proj.astype(np.float64)
    return acc.reshape(B, P, D).astype(np.float32)
